# revision 31
# baseline (speedup 1.0000x reference)
"""Channel-attention (CAM) Bass kernel for TRN2, SPMD over 8 NeuronCores.

Computes, for each batch b:
    A   = inputs[b].reshape(HW, C)
    G   = A.T @ A                      (Gram, [C, C])
    S   = softmax(G, axis=-1)
    out = gamma * (A @ S) + A

Sharding: data-parallel over batch. 16 batches / 8 cores = 2 batches per core.

Numerics: the epilogue is computed in residual form
    out = A @ (gamma*S - gamma*I) + (1 + gamma) * A
which is algebraically identical but applies the identity component of S to
the exact fp32 copy of A, so the fp8 matmul precision only touches the
gamma*(S - I) term.

Design (fp8 DoubleRow, staggered 2-batch pipeline; ~120 us vs 163 us for
the bf16 baseline):
  - Row-pair layout: SBUF chunk [128, 2, 512] holds rows 256t + 2p + r on
    partition p; chunks are loaded/stored two at a time (2 MiB DMAs, 4 KiB
    contiguous descriptors per partition; ~400 GB/s observed vs ~325 GB/s
    with the row-per-partition 2 KiB layout).  The pair dimension is also
    exactly the fp8 DoubleRow k-subtile pair, so Gram contracts 256 rows
    per instruction at 2x bf16 rate.
  - A cast to fp8e4 on ScalarE (one chunk of lookahead so the ScalarE
    stream never serializes against the PE round-trip).
  - A^T for the attend is built with "dual transposes": a regular fp8
    DoubleRow matmul against a block-diagonal double identity constant
    transposes two adjacent 128x128 blocks per instruction; PSUM->SBUF
    copies split between DVE and ScalarE.
  - Softmax: DVE row-max (negated) -> ScalarE Exp with accum_out row-sum
    -> DVE reciprocal -> scale by gamma -> S'' = (E * gamma*r) - gamma*I
    written as fp8e4 into the DoubleRow-paired S2 tiles.
  - Attend: DoubleRow matmuls, one PSUM bank per (chunk, r); epilogue
    out = psum + (1+gamma)*raw as one DVE scalar_tensor_tensor per r-half.
  - Schedule: phase A loads batch0 with its full Gram + transposes inline
    (DMA-paced); batch1's load streams right behind; phase C interleaves
    batch0's attend with batch1's cast + transposes + half its Gram; the
    rest of batch1's Gram runs as a dense m-major PE burst so each softmax
    chain starts as soon as its accumulator closes; phase D is batch1's
    attend with 4-deep PSUM buffering (transpose and attend banks share
    one pool), per-chunk stores at the end to shorten the DMA tail.
  - PSUM: 4 Gram banks + 4 shared transpose/attend banks = 8.
"""

import numpy as np

import concourse.bass as bass
import concourse.mybir as mybir
import concourse.tile as tile
from concourse import bacc
from concourse.bass import ds, ts
from concourse.masks import make_identity

P = 128
N_CORES = 8
B_TOTAL = 16
B_PER_CORE = B_TOTAL // N_CORES  # 2
H = 64
W = 64
HW = H * W          # 4096
C = 512
R = 2               # rows per partition (DoubleRow pair)
NT = HW // (P * R)  # 16 chunks of 256 rows per batch
NT2 = NT // 2       # 8 chunk pairs
M = C // P          # 4 channel chunks
MP = M // 2         # 2 channel-chunk pairs

F32 = mybir.dt.float32
FP8 = mybir.dt.float8e4
AX = mybir.AxisListType
ALU = mybir.AluOpType
ACT_FN = mybir.ActivationFunctionType
DR = mybir.MatmulPerfMode.DoubleRow


class _BatchState:
    __slots__ = ("a_b", "o_b", "raw2", "a8", "at", "g_ps", "s2", "o2")

    def __init__(self, a_b, o_b):
        self.a_b = a_b
        self.o_b = o_b
        self.raw2 = []   # pair tiles [P, 2, R, C]
        self.a8 = {}
        self.at = []
        self.g_ps = {}
        self.s2 = None
        self.o2 = None

    def raw(self, t2):
        return self.raw2[t2 // 2][:, t2 % 2]


def _build_kernel(tc, a_dram, gamma_dram, o_dram):
    nc = tc.nc
    from contextlib import ExitStack

    with ExitStack() as ctx:
        const_pool = ctx.enter_context(tc.tile_pool(name="const", bufs=1))
        raw_pool = ctx.enter_context(tc.tile_pool(name="raw", bufs=15))
        a8_pool = ctx.enter_context(tc.tile_pool(name="a8", bufs=18))
        at_pool = ctx.enter_context(tc.tile_pool(name="at", bufs=17))
        e_pool = ctx.enter_context(tc.tile_pool(name="e", bufs=2))
        s_pool = ctx.enter_context(tc.tile_pool(name="s", bufs=2 * MP))
        st_pool = ctx.enter_context(tc.tile_pool(name="st", bufs=16))
        o_pool = ctx.enter_context(tc.tile_pool(name="o", bufs=4))
        pg_pool = ctx.enter_context(tc.tile_pool(name="pg", bufs=4, space="PSUM"))
        ps_pool = ctx.enter_context(tc.tile_pool(name="ps", bufs=4, space="PSUM"))

        # Dual identity for 2-at-a-time block transposes: I2[p, 0, f] = [f==p],
        # I2[p, 1, f] = [f==128+p]  (fp8; 1.0 is exact).
        ident2 = const_pool.tile([P, R, 2 * P], FP8, tag="ident2")
        i2flat = ident2.rearrange("p q f -> p (q f)")
        nc.gpsimd.memset(i2flat, 0.0)
        nc.gpsimd.affine_select(
            out=i2flat, in_=i2flat, compare_op=ALU.not_equal, fill=1.0,
            base=0, pattern=[[-1, 2 * R * P]], channel_multiplier=1,
        )
        nc.gpsimd.affine_select(
            out=i2flat, in_=i2flat, compare_op=ALU.not_equal, fill=1.0,
            base=3 * P, pattern=[[-1, 2 * R * P]], channel_multiplier=1,
        )

        gamma_sb = const_pool.tile([P, 1], F32, tag="gamma")
        nc.sync.dma_start(gamma_sb, gamma_dram)
        gamma2_sb = const_pool.tile([P, 1], F32, tag="gamma2")
        nc.vector.tensor_scalar_add(gamma2_sb, gamma_sb, 1.0)
        # identrow[m]: gamma * I placed at columns [128m, 128m+128) of a
        # [128, 512] row block, fp32
        identrow = []
        for m in range(M):
            ir = const_pool.tile([P, C], F32, tag=f"identrow{m}", name="ir")
            nc.gpsimd.memset(ir, 0.0)
            make_identity(nc, ir[:, ts(m, P)], nomemset=True)
            nc.vector.tensor_scalar_mul(ir, ir, gamma_sb)
            identrow.append(ir)

        bs = []
        for b in range(B_PER_CORE):
            # row = 256*(2j+u) + 2*p + r on partition p; chunk pairs share DMAs
            bs.append(
                _BatchState(
                    a_dram[b].rearrange("(j u p r) c -> p j u r c", p=P, r=R, u=2),
                    o_dram[b].rearrange("(j u p r) c -> p j u r c", p=P, r=R, u=2),
                )
            )

        # ---- emission helpers -------------------------------------------

        def load_pair(s, j):
            r2 = raw_pool.tile([P, 2, R, C], F32, tag="raw", name="r2")
            if j == 0:
                # finer split so the first Gram chunk starts ASAP
                for r in range(R):
                    nc.sync.dma_start(r2[:, 0, r, :], s.a_b[:, 0, 0, r, :])
                nc.sync.dma_start(r2[:, 1], s.a_b[:, 0, 1])
            else:
                nc.sync.dma_start(r2, s.a_b[:, j])
            s.raw2.append(r2)

        def cast_chunk(s, t2):
            a8t = a8_pool.tile([P, R, M, P], FP8, tag="a8", name="a8t")
            a8v = a8t.rearrange("p r m n -> p (r m n)")
            rawv = s.raw(t2).rearrange("p r c -> p (r c)")
            if t2 == 0:
                half = M * P
                nc.scalar.activation(
                    a8v[:, :half], rawv[:, :half], ACT_FN.Copy, bias=0.0, scale=1.0
                )
                nc.scalar.activation(
                    a8v[:, half:], rawv[:, half:], ACT_FN.Copy, bias=0.0, scale=1.0
                )
            else:
                nc.scalar.activation(a8v, rawv, ACT_FN.Copy, bias=0.0, scale=1.0)
            s.a8[t2] = a8t

        def gram_chunk(s, t2):
            for m in range(M):
                nc.tensor.matmul(
                    s.g_ps[m],
                    s.a8[t2][:, :, m, :],
                    s.a8[t2].rearrange("p r m n -> p r (m n)"),
                    start=(t2 == 0),
                    stop=(t2 == NT - 1),
                    perf_mode=DR,
                )

        def alloc_gram(s):
            for m in range(M):
                s.g_ps[m] = pg_pool.tile([P, C], F32, tag="pg", name="g_ps")

        def dual_transposes(s, t2, copy_engs):
            att = at_pool.tile([P, MP, R, 2, P], FP8, tag="at", name="att")
            for mp in range(MP):
                pt = ps_pool.tile([P, C], F32, tag="ps", name="pt").rearrange(
                    "p (r f) -> p r f", r=R
                )
                for r in range(R):
                    nc.tensor.matmul(
                        pt[:, r, :],
                        s.a8[t2][:, r, ts(mp, 2), :],
                        ident2,
                        start=True,
                        stop=True,
                        perf_mode=DR,
                    )
                dst = att[:, mp].rearrange("p r q n -> p (r q n)")
                src = pt.rearrange("p r f -> p (r f)")
                if copy_engs[mp] == "v":
                    nc.vector.tensor_copy(out=dst, in_=src)
                else:
                    nc.scalar.activation(dst, src, ACT_FN.Copy, bias=0.0, scale=1.0)
            s.at.append(att)

        def softmax(s):
            s.s2 = [
                s_pool.tile([P, R, C], FP8, tag="s2", name="s2") for _ in range(MP)
            ]
            for m in range(M):
                negmax = st_pool.tile([P, 1], F32, tag="stat", name="negmax")
                nc.vector.tensor_reduce(
                    negmax, s.g_ps[m], axis=AX.X, op=ALU.max, negate=True
                )
                e = e_pool.tile([P, C], F32, tag="e", name="e")
                dsum = st_pool.tile([P, 1], F32, tag="stat", name="dsum")
                nc.scalar.activation(
                    e, s.g_ps[m], ACT_FN.Exp, bias=negmax, scale=1.0, accum_out=dsum
                )
                r_ = st_pool.tile([P, 1], F32, tag="stat", name="r")
                nc.vector.reciprocal(r_, dsum)
                r2 = st_pool.tile([P, 1], F32, tag="stat", name="r2")
                nc.vector.tensor_scalar_mul(r2, r_, gamma_sb)
                nc.vector.scalar_tensor_tensor(
                    s.s2[m // 2][:, m % 2, :], e, r2, identrow[m],
                    op0=ALU.mult, op1=ALU.subtract,
                )

        def attend_chunk(s, t2, split_out=False):
            if t2 % 2 == 0:
                s.o2 = o_pool.tile([P, 2, R, C], F32, tag="o", name="o2")
            o_sb = s.o2[:, t2 % 2]
            for r in range(R):
                o_ps = ps_pool.tile([P, C], F32, tag="ps", name="o_ps")
                for mp in range(MP):
                    nc.tensor.matmul(
                        o_ps,
                        s.at[t2][:, mp, r, :, :],
                        s.s2[mp],
                        start=(mp == 0),
                        stop=(mp == MP - 1),
                        perf_mode=DR,
                    )
                nc.vector.scalar_tensor_tensor(
                    o_sb[:, r, :], s.raw(t2)[:, r, :], gamma2_sb, o_ps,
                    op0=ALU.mult, op1=ALU.add,
                )
            if split_out:
                # per-chunk store to shorten the final DMA tail
                nc.sync.dma_start(s.o_b[:, t2 // 2, t2 % 2], o_sb)
            elif t2 % 2 == 1:
                nc.sync.dma_start(s.o_b[:, t2 // 2], s.o2)

        # ---- schedule ----------------------------------------------------

        b0, b1 = bs

        # Phase A: load batch0; full Gram + dual transposes inline.  The
        # cast for chunk t2+1 is emitted ahead of chunk t2's at-copies so
        # the ScalarE stream never serializes against the PE round-trip.
        alloc_gram(b0)
        load_pair(b0, 0)
        cast_chunk(b0, 0)
        for t2 in range(NT):
            if t2 % 2 == 1 and t2 + 1 < NT:
                load_pair(b0, (t2 + 1) // 2)
            if t2 + 1 < NT:
                cast_chunk(b0, t2 + 1)
            if t2 < NT - 2:
                gram_chunk(b0, t2)
            dual_transposes(b0, t2, ("v", "v"))
        # last two chunks m-major so each softmax chain starts as soon as
        # its accumulator closes
        for m in range(M):
            for t2 in (NT - 2, NT - 1):
                nc.tensor.matmul(
                    b0.g_ps[m],
                    b0.a8[t2][:, :, m, :],
                    b0.a8[t2].rearrange("p r m n -> p r (m n)"),
                    start=False,
                    stop=(t2 == NT - 1),
                    perf_mode=DR,
                )
        for j in range(NT2):
            load_pair(b1, j)
        cast_chunk(b1, 0)
        softmax(b0)

        # Phase C: batch0 attend interleaved with batch1 cast + full Gram +
        # dual transposes (batch1 data streams in during this window).
        # b1-prep precedes the attend in the PE stream so the transposes for
        # chunk 0 fill the softmax latency.
        alloc_gram(b1)
        for t2 in range(NT):
            if t2 + 1 < NT:
                cast_chunk(b1, t2 + 1)
            if t2 < NT // 2:
                dual_transposes(b1, t2, ("v", "s"))
            else:
                gram_chunk(b1, t2 - NT // 2)
            attend_chunk(b0, t2, split_out=True)
        # rest of batch1's Gram as a dense PE burst, m-major so each
        # softmax chain starts as soon as its accumulator closes
        for m in range(M):
            for t2 in range(NT // 2, NT):
                nc.tensor.matmul(
                    b1.g_ps[m],
                    b1.a8[t2][:, :, m, :],
                    b1.a8[t2].rearrange("p r m n -> p r (m n)"),
                    start=False,
                    stop=(t2 == NT - 1),
                    perf_mode=DR,
                )
        softmax(b1)

        # Phase D: batch1 attend; the second half of its transposes run
        # here, where ScalarE is otherwise idle
        for t2 in range(NT):
            if t2 < NT // 2:
                dual_transposes(b1, t2 + NT // 2, ("s", "s"))
            attend_chunk(b1, t2, split_out=True)


_NC_CACHE = None


def build():
    global _NC_CACHE
    if _NC_CACHE is not None:
        return _NC_CACHE
    nc = bacc.Bacc(
        "TRN2",
        target_bir_lowering=False,
        debug=False,
        enable_asserts=False,
        num_devices=N_CORES,
    )
    a_dram = nc.dram_tensor("a", [B_PER_CORE, HW, C], F32, kind="ExternalInput").ap()
    gamma_dram = nc.dram_tensor("gamma", [P, 1], F32, kind="ExternalInput").ap()
    o_dram = nc.dram_tensor("o", [B_PER_CORE, HW, C], F32, kind="ExternalOutput").ap()
    with tile.TileContext(nc) as tc:
        _build_kernel(tc, a_dram, gamma_dram, o_dram)
    nc.compile()
    _NC_CACHE = nc
    return nc


def make_in_maps(inputs, gamma):
    x = np.ascontiguousarray(np.asarray(inputs, dtype=np.float32)).reshape(
        B_TOTAL, HW, C
    )
    gb = np.ascontiguousarray(
        np.broadcast_to(np.asarray(gamma, dtype=np.float32).reshape(1, 1), (P, 1))
    )
    return [
        {"a": x[i * B_PER_CORE : (i + 1) * B_PER_CORE], "gamma": gb}
        for i in range(N_CORES)
    ]


def run(inputs, gamma, trace=False, **kw):
    from concourse import bass_utils

    nc = build()
    in_maps = make_in_maps(inputs, gamma)
    res = bass_utils.run_bass_kernel_spmd(
        nc, in_maps, core_ids=list(range(N_CORES)), trace=trace, **kw
    )
    out = np.concatenate([r["o"] for r in res.results], axis=0)
    return out.reshape(B_TOTAL, H, W, C).astype(np.float32, copy=False), res


def kernel(inputs, gamma):
    out, _ = run(inputs, gamma, trace=False)
    return out


# revision 34
# speedup vs baseline: 1.0501x; 1.0501x over previous
"""Channel-attention (CAM) Bass kernel for TRN2, SPMD over 8 NeuronCores.

Computes, for each batch b:
    A   = inputs[b].reshape(HW, C)
    G   = A.T @ A                      (Gram, [C, C])
    S   = softmax(G, axis=-1)
    out = gamma * (A @ S) + A

Sharding: data-parallel over batch. 16 batches / 8 cores = 2 batches per core.

Numerics: the epilogue is computed in residual form
    out = A @ (gamma*S - gamma*I) + (1 + gamma) * A
which is algebraically identical but applies the identity component of S to
the exact fp32 copy of A, so the fp8 matmul precision only touches the
gamma*(S - I) term.

Design (fp8 DoubleRow, staggered 2-batch pipeline; ~120 us vs 163 us for
the bf16 baseline):
  - Row-pair layout: SBUF chunk [128, 2, 512] holds rows 256t + 2p + r on
    partition p; chunks are loaded/stored two at a time (2 MiB DMAs, 4 KiB
    contiguous descriptors per partition; ~400 GB/s observed vs ~325 GB/s
    with the row-per-partition 2 KiB layout).  The pair dimension is also
    exactly the fp8 DoubleRow k-subtile pair, so Gram contracts 256 rows
    per instruction at 2x bf16 rate.
  - A cast to fp8e4 on ScalarE (one chunk of lookahead so the ScalarE
    stream never serializes against the PE round-trip).
  - A^T for the attend is built with "dual transposes": a regular fp8
    DoubleRow matmul against a block-diagonal double identity constant
    transposes two adjacent 128x128 blocks per instruction; PSUM->SBUF
    copies split between DVE and ScalarE.
  - Softmax: DVE row-max (negated) -> ScalarE Exp with accum_out row-sum
    -> DVE reciprocal -> scale by gamma -> S'' = (E * gamma*r) - gamma*I
    written as fp8e4 into the DoubleRow-paired S2 tiles.
  - Attend: DoubleRow matmuls, one PSUM bank per (chunk, r); epilogue
    out = psum + (1+gamma)*raw as one DVE scalar_tensor_tensor per r-half.
  - Schedule: phase A loads batch0 with its full Gram + transposes inline
    (DMA-paced); batch1's load streams right behind; phase C interleaves
    batch0's attend with batch1's cast + transposes + half its Gram; the
    rest of batch1's Gram runs as a dense m-major PE burst so each softmax
    chain starts as soon as its accumulator closes; phase D is batch1's
    attend with 4-deep PSUM buffering (transpose and attend banks share
    one pool) plus the second half of its transposes (ScalarE is idle
    there).  Attend phases store per-chunk (1 MiB DMAs) so the output
    stream starts early and drains smoothly.
  - PSUM: 4 Gram banks + 4 shared transpose/attend banks = 8.
"""

import numpy as np

import concourse.bass as bass
import concourse.mybir as mybir
import concourse.tile as tile
from concourse import bacc
from concourse.bass import ds, ts
from concourse.masks import make_identity

P = 128
N_CORES = 8
B_TOTAL = 16
B_PER_CORE = B_TOTAL // N_CORES  # 2
H = 64
W = 64
HW = H * W          # 4096
C = 512
R = 2               # rows per partition (DoubleRow pair)
NT = HW // (P * R)  # 16 chunks of 256 rows per batch
NT2 = NT // 2       # 8 chunk pairs
M = C // P          # 4 channel chunks
MP = M // 2         # 2 channel-chunk pairs

F32 = mybir.dt.float32
FP8 = mybir.dt.float8e4
AX = mybir.AxisListType
ALU = mybir.AluOpType
ACT_FN = mybir.ActivationFunctionType
DR = mybir.MatmulPerfMode.DoubleRow


class _BatchState:
    __slots__ = ("a_b", "o_b", "raw2", "a8", "at", "g_ps", "s2", "o2")

    def __init__(self, a_b, o_b):
        self.a_b = a_b
        self.o_b = o_b
        self.raw2 = []   # pair tiles [P, 2, R, C]
        self.a8 = {}
        self.at = []
        self.g_ps = {}
        self.s2 = None
        self.o2 = None

    def raw(self, t2):
        return self.raw2[t2 // 2][:, t2 % 2]


def _build_kernel(tc, a_dram, gamma_dram, o_dram):
    nc = tc.nc
    from contextlib import ExitStack

    with ExitStack() as ctx:
        const_pool = ctx.enter_context(tc.tile_pool(name="const", bufs=1))
        raw_pool = ctx.enter_context(tc.tile_pool(name="raw", bufs=15))
        a8_pool = ctx.enter_context(tc.tile_pool(name="a8", bufs=18))
        at_pool = ctx.enter_context(tc.tile_pool(name="at", bufs=18))
        e_pool = ctx.enter_context(tc.tile_pool(name="e", bufs=2))
        s_pool = ctx.enter_context(tc.tile_pool(name="s", bufs=2 * MP))
        st_pool = ctx.enter_context(tc.tile_pool(name="st", bufs=16))
        o_pool = ctx.enter_context(tc.tile_pool(name="o", bufs=4))
        pg_pool = ctx.enter_context(tc.tile_pool(name="pg", bufs=4, space="PSUM"))
        ps_pool = ctx.enter_context(tc.tile_pool(name="ps", bufs=4, space="PSUM"))

        # Dual identity for 2-at-a-time block transposes: I2[p, 0, f] = [f==p],
        # I2[p, 1, f] = [f==128+p]  (fp8; 1.0 is exact).
        ident2 = const_pool.tile([P, R, 2 * P], FP8, tag="ident2")
        i2flat = ident2.rearrange("p q f -> p (q f)")
        nc.gpsimd.memset(i2flat, 0.0)
        nc.gpsimd.affine_select(
            out=i2flat, in_=i2flat, compare_op=ALU.not_equal, fill=1.0,
            base=0, pattern=[[-1, 2 * R * P]], channel_multiplier=1,
        )
        nc.gpsimd.affine_select(
            out=i2flat, in_=i2flat, compare_op=ALU.not_equal, fill=1.0,
            base=3 * P, pattern=[[-1, 2 * R * P]], channel_multiplier=1,
        )

        gamma_sb = const_pool.tile([P, 1], F32, tag="gamma")
        nc.sync.dma_start(gamma_sb, gamma_dram)
        gamma2_sb = const_pool.tile([P, 1], F32, tag="gamma2")
        nc.vector.tensor_scalar_add(gamma2_sb, gamma_sb, 1.0)
        # identrow[m]: gamma * I placed at columns [128m, 128m+128) of a
        # [128, 512] row block, fp32
        identrow = []
        for m in range(M):
            ir = const_pool.tile([P, C], F32, tag=f"identrow{m}", name="ir")
            nc.gpsimd.memset(ir, 0.0)
            make_identity(nc, ir[:, ts(m, P)], nomemset=True)
            nc.vector.tensor_scalar_mul(ir, ir, gamma_sb)
            identrow.append(ir)

        bs = []
        for b in range(B_PER_CORE):
            # row = 256*(2j+u) + 2*p + r on partition p; chunk pairs share DMAs
            bs.append(
                _BatchState(
                    a_dram[b].rearrange("(j u p r) c -> p j u r c", p=P, r=R, u=2),
                    o_dram[b].rearrange("(j u p r) c -> p j u r c", p=P, r=R, u=2),
                )
            )

        # ---- emission helpers -------------------------------------------

        def load_pair(s, j):
            r2 = raw_pool.tile([P, 2, R, C], F32, tag="raw", name="r2")
            if j == 0:
                # finer split so the first Gram chunk starts ASAP
                for r in range(R):
                    nc.sync.dma_start(r2[:, 0, r, :], s.a_b[:, 0, 0, r, :])
                nc.sync.dma_start(r2[:, 1], s.a_b[:, 0, 1])
            else:
                nc.sync.dma_start(r2, s.a_b[:, j])
            s.raw2.append(r2)

        def cast_chunk(s, t2):
            a8t = a8_pool.tile([P, R, M, P], FP8, tag="a8", name="a8t")
            a8v = a8t.rearrange("p r m n -> p (r m n)")
            rawv = s.raw(t2).rearrange("p r c -> p (r c)")
            if t2 == 0:
                half = M * P
                nc.scalar.activation(
                    a8v[:, :half], rawv[:, :half], ACT_FN.Copy, bias=0.0, scale=1.0
                )
                nc.scalar.activation(
                    a8v[:, half:], rawv[:, half:], ACT_FN.Copy, bias=0.0, scale=1.0
                )
            else:
                nc.scalar.activation(a8v, rawv, ACT_FN.Copy, bias=0.0, scale=1.0)
            s.a8[t2] = a8t

        def gram_chunk(s, t2):
            for m in range(M):
                nc.tensor.matmul(
                    s.g_ps[m],
                    s.a8[t2][:, :, m, :],
                    s.a8[t2].rearrange("p r m n -> p r (m n)"),
                    start=(t2 == 0),
                    stop=(t2 == NT - 1),
                    perf_mode=DR,
                )

        def alloc_gram(s):
            for m in range(M):
                s.g_ps[m] = pg_pool.tile([P, C], F32, tag="pg", name="g_ps")

        def dual_transposes(s, t2, copy_engs):
            att = at_pool.tile([P, MP, R, 2, P], FP8, tag="at", name="att")
            for mp in range(MP):
                pt = ps_pool.tile([P, C], F32, tag="ps", name="pt").rearrange(
                    "p (r f) -> p r f", r=R
                )
                for r in range(R):
                    nc.tensor.matmul(
                        pt[:, r, :],
                        s.a8[t2][:, r, ts(mp, 2), :],
                        ident2,
                        start=True,
                        stop=True,
                        perf_mode=DR,
                    )
                dst = att[:, mp].rearrange("p r q n -> p (r q n)")
                src = pt.rearrange("p r f -> p (r f)")
                if copy_engs[mp] == "v":
                    nc.vector.tensor_copy(out=dst, in_=src)
                else:
                    nc.scalar.activation(dst, src, ACT_FN.Copy, bias=0.0, scale=1.0)
            s.at.append(att)

        def softmax(s):
            s.s2 = [
                s_pool.tile([P, R, C], FP8, tag="s2", name="s2") for _ in range(MP)
            ]
            for m in range(M):
                negmax = st_pool.tile([P, 1], F32, tag="stat", name="negmax")
                nc.vector.tensor_reduce(
                    negmax, s.g_ps[m], axis=AX.X, op=ALU.max, negate=True
                )
                e = e_pool.tile([P, C], F32, tag="e", name="e")
                dsum = st_pool.tile([P, 1], F32, tag="stat", name="dsum")
                nc.scalar.activation(
                    e, s.g_ps[m], ACT_FN.Exp, bias=negmax, scale=1.0, accum_out=dsum
                )
                r_ = st_pool.tile([P, 1], F32, tag="stat", name="r")
                nc.vector.reciprocal(r_, dsum)
                r2 = st_pool.tile([P, 1], F32, tag="stat", name="r2")
                nc.vector.tensor_scalar_mul(r2, r_, gamma_sb)
                nc.vector.scalar_tensor_tensor(
                    s.s2[m // 2][:, m % 2, :], e, r2, identrow[m],
                    op0=ALU.mult, op1=ALU.subtract,
                )

        def attend_chunk(s, t2, split_out=False):
            if t2 % 2 == 0:
                s.o2 = o_pool.tile([P, 2, R, C], F32, tag="o", name="o2")
            o_sb = s.o2[:, t2 % 2]
            for r in range(R):
                o_ps = ps_pool.tile([P, C], F32, tag="ps", name="o_ps")
                for mp in range(MP):
                    nc.tensor.matmul(
                        o_ps,
                        s.at[t2][:, mp, r, :, :],
                        s.s2[mp],
                        start=(mp == 0),
                        stop=(mp == MP - 1),
                        perf_mode=DR,
                    )
                nc.vector.scalar_tensor_tensor(
                    o_sb[:, r, :], s.raw(t2)[:, r, :], gamma2_sb, o_ps,
                    op0=ALU.mult, op1=ALU.add,
                )
            if split_out:
                # per-chunk store to shorten the final DMA tail
                nc.sync.dma_start(s.o_b[:, t2 // 2, t2 % 2], o_sb)
            elif t2 % 2 == 1:
                nc.sync.dma_start(s.o_b[:, t2 // 2], s.o2)

        # ---- schedule ----------------------------------------------------

        b0, b1 = bs

        # Phase A: load batch0; full Gram + dual transposes inline.  The
        # cast for chunk t2+1 is emitted ahead of chunk t2's at-copies so
        # the ScalarE stream never serializes against the PE round-trip.
        alloc_gram(b0)
        load_pair(b0, 0)
        cast_chunk(b0, 0)
        for t2 in range(NT):
            if t2 % 2 == 1 and t2 + 1 < NT:
                load_pair(b0, (t2 + 1) // 2)
            if t2 + 1 < NT:
                cast_chunk(b0, t2 + 1)
            if t2 < NT - 2:
                gram_chunk(b0, t2)
            dual_transposes(b0, t2, ("v", "v"))
        # last two chunks m-major so each softmax chain starts as soon as
        # its accumulator closes
        for m in range(M):
            for t2 in (NT - 2, NT - 1):
                nc.tensor.matmul(
                    b0.g_ps[m],
                    b0.a8[t2][:, :, m, :],
                    b0.a8[t2].rearrange("p r m n -> p r (m n)"),
                    start=False,
                    stop=(t2 == NT - 1),
                    perf_mode=DR,
                )
        for j in range(NT2):
            load_pair(b1, j)
        cast_chunk(b1, 0)
        softmax(b0)

        # Phase C: batch0 attend interleaved with batch1 cast + full Gram +
        # dual transposes (batch1 data streams in during this window).
        # b1-prep precedes the attend in the PE stream so the transposes for
        # chunk 0 fill the softmax latency.
        alloc_gram(b1)
        for t2 in range(NT):
            if t2 + 1 < NT:
                cast_chunk(b1, t2 + 1)
            if t2 < NT // 2:
                dual_transposes(b1, t2, ("v", "s"))
            else:
                gram_chunk(b1, t2 - NT // 2)
            attend_chunk(b0, t2, split_out=True)
        # rest of batch1's Gram as a dense PE burst, m-major so each
        # softmax chain starts as soon as its accumulator closes
        for m in range(M):
            for t2 in range(NT // 2, NT):
                nc.tensor.matmul(
                    b1.g_ps[m],
                    b1.a8[t2][:, :, m, :],
                    b1.a8[t2].rearrange("p r m n -> p r (m n)"),
                    start=False,
                    stop=(t2 == NT - 1),
                    perf_mode=DR,
                )
        softmax(b1)

        # Phase D: batch1 attend; the second half of its transposes run
        # here, where ScalarE is otherwise idle
        for t2 in range(NT):
            if t2 < NT // 2:
                dual_transposes(b1, t2 + NT // 2, ("s", "s"))
            attend_chunk(b1, t2, split_out=True)


_NC_CACHE = None


def build():
    global _NC_CACHE
    if _NC_CACHE is not None:
        return _NC_CACHE
    nc = bacc.Bacc(
        "TRN2",
        target_bir_lowering=False,
        debug=False,
        enable_asserts=False,
        num_devices=N_CORES,
    )
    a_dram = nc.dram_tensor("a", [B_PER_CORE, HW, C], F32, kind="ExternalInput").ap()
    gamma_dram = nc.dram_tensor("gamma", [P, 1], F32, kind="ExternalInput").ap()
    o_dram = nc.dram_tensor("o", [B_PER_CORE, HW, C], F32, kind="ExternalOutput").ap()
    with tile.TileContext(nc) as tc:
        _build_kernel(tc, a_dram, gamma_dram, o_dram)
    nc.compile()
    _NC_CACHE = nc
    return nc


def make_in_maps(inputs, gamma):
    x = np.ascontiguousarray(np.asarray(inputs, dtype=np.float32)).reshape(
        B_TOTAL, HW, C
    )
    gb = np.ascontiguousarray(
        np.broadcast_to(np.asarray(gamma, dtype=np.float32).reshape(1, 1), (P, 1))
    )
    return [
        {"a": x[i * B_PER_CORE : (i + 1) * B_PER_CORE], "gamma": gb}
        for i in range(N_CORES)
    ]


def run(inputs, gamma, trace=False, **kw):
    from concourse import bass_utils

    nc = build()
    in_maps = make_in_maps(inputs, gamma)
    res = bass_utils.run_bass_kernel_spmd(
        nc, in_maps, core_ids=list(range(N_CORES)), trace=trace, **kw
    )
    out = np.concatenate([r["o"] for r in res.results], axis=0)
    return out.reshape(B_TOTAL, H, W, C).astype(np.float32, copy=False), res


def kernel(inputs, gamma):
    out, _ = run(inputs, gamma, trace=False)
    return out
